# Initial kernel scaffold
#
"""Causal multi-head attention with RoPE for Trainium2, sharded over 8 NeuronCores.

Problem: B=4, T=2048, C=768, H=12, D=64, fp32 in/out.
    q,k,v = x @ wq/wk/wv  (per-head reshape), RoPE(q,k), causal softmax(q k^T/sqrt(D)) v,
    out = concat_heads @ wo.

Sharding: core c -> (batch b = c//2, head-group g = c%2 covering heads g*6..g*6+5).
Each core computes its 6 heads' attention and a partial output projection
y_c = out_heads(g) @ wo[rows g]; the host sums the two partials per batch.

On-core dataflow (bf16 matmul operands, fp32 PSUM accumulation; bf16 enables
Fast Weight Load and 1024-wide moving operands, so the q/k projection and
rotation chains use half the matmul instructions):
  - host passes x^T (bf16); input DMAs split across all three DGE queues,
    critical tensors first; a HAM-warmup matmul stream covers the load.
  - q^T,k^T in [head_dim, T] layout; RoPE via a block-rotation matmul +
    cos/sin tensor ops on DVE, pipelined one block behind the projection
    so the PSUM->SBUF copy latency never stalls the PE.
  - scores transposed: S^T[k, q] = k^T.T @ q^T with K=64 row-pairing
    (even head at partitions 0:64, odd at 64:128 -> concurrent row groups).
    The first two (p=0, qc=3) score groups are emitted inside the phase-1
    tail and the rest zip against the v-projection, so the ScalarE exp
    stream (the second serial resource, ~13.4M exps/core) starts the
    moment phase 1 drains.
  - P = exp(S/8) on ScalarE -> bf16; causal masking of diagonal tiles via
    a bf16 utri@eband matmul add before exp.
  - PV with a ones-row appended to V: out_unnorm^T[d, q] and l[q] in one
    accumulated matmul chain per (head, q-chunk); 1-group score lookahead
    keeps the PE ahead of the exp stream.
  - normalization: l row -> partition 0, gpsimd.partition_broadcast,
    reciprocal_approx_fast, TT multiply.  The small qc=1/qc=0 tails run
    breadth-first across head-pairs with their PSUM->SBUF copies moved to
    the by-then idle ScalarE.
  - output projection accumulates 3 head-pair chunks into [128, 768] PSUM.
"""

import numpy as np
from contextlib import ExitStack

B, T, C, H, D = 4, 2048, 768, 12, 64
HPC = 6          # heads per core
NP = 3           # head-pair tiles per core
CC = C // 128    # 6 contraction chunks
TT = T // 128    # 16 t tiles
QC = T // 512    # 4 q chunks
KC = T // 128    # 16 k chunks

_COMPILED = None


def _rope_tables():
    import ml_dtypes
    inv_freq = 1.0 / (10000.0 ** (np.arange(0, D, 2, dtype=np.float64) / D))  # [32]
    t = np.arange(T, dtype=np.float64)
    freqs = np.outer(t, inv_freq)                      # [T, 32]
    cosT = np.cos(freqs).T.astype(np.float32)          # [32, T]
    sinT = np.sin(freqs).T.astype(np.float32)
    ccat = np.tile(cosT, (4, 1)).astype(ml_dtypes.bfloat16)   # [128, T]
    scat = np.tile(sinT, (4, 1)).astype(ml_dtypes.bfloat16)
    return np.ascontiguousarray(ccat), np.ascontiguousarray(scat)


def _rot_matrix():
    import ml_dtypes
    # rotate_half as a matmul: rot = R @ q (q in [D, T] layout), per 64-row block
    R = np.zeros((D, D), dtype=np.float32)
    R[0:32, 32:64] = -np.eye(32, dtype=np.float32)
    R[32:64, 0:32] = np.eye(32, dtype=np.float32)
    R2 = np.zeros((128, 128), dtype=np.float32)
    R2[0:64, 0:64] = R
    R2[64:128, 64:128] = R
    return np.ascontiguousarray(R2.T.astype(ml_dtypes.bfloat16))  # lhsT for out = R2 @ q


def _build_program():
    import concourse.tile as tile
    from concourse import bacc, mybir

    F32 = mybir.dt.float32
    BF16 = mybir.dt.bfloat16
    EXP = mybir.ActivationFunctionType.Exp

    nc = bacc.Bacc("TRN2", target_bir_lowering=False, debug=False, num_devices=8)

    xT_d = nc.dram_tensor("xT", [C, T], BF16, kind="ExternalInput").ap()
    wq_d = nc.dram_tensor("wq", [C, HPC * D], BF16, kind="ExternalInput").ap()
    wk_d = nc.dram_tensor("wk", [C, HPC * D], BF16, kind="ExternalInput").ap()
    wv_d = nc.dram_tensor("wv", [C, HPC * D], BF16, kind="ExternalInput").ap()
    wo_d = nc.dram_tensor("wo", [HPC * D, C], BF16, kind="ExternalInput").ap()
    ccat_d = nc.dram_tensor("ccat", [128, T], BF16, kind="ExternalInput").ap()
    scat_d = nc.dram_tensor("scat", [128, T], BF16, kind="ExternalInput").ap()
    r2t_d = nc.dram_tensor("r2t", [128, 128], BF16, kind="ExternalInput").ap()
    utri_d = nc.dram_tensor("utri", [128, 128], BF16, kind="ExternalInput").ap()
    eband_d = nc.dram_tensor("eband", [128, 128], BF16, kind="ExternalInput").ap()
    y_d = nc.dram_tensor("y", [T, C], F32, kind="ExternalOutput").ap()

    with tile.TileContext(nc) as tc, ExitStack() as ctx:
        big_pool = ctx.enter_context(tc.tile_pool(name="big", bufs=1))
        q_all = big_pool.tile([128, NP, T], BF16)
        k_all = big_pool.tile([128, NP, T], BF16)
        v_aug = big_pool.tile([128, KC, HPC, D + 1], BF16)
        out_norm = big_pool.tile([128, NP, T], BF16)

        cst_pool = ctx.enter_context(tc.tile_pool(name="cst", bufs=1))
        xt_pool = ctx.enter_context(tc.tile_pool(name="xt", bufs=1))
        xt_sb = xt_pool.tile([128, CC, T], BF16)

        p_sbp = ctx.enter_context(tc.tile_pool(name="p_sb", bufs=12))
        l_sbp = ctx.enter_context(tc.tile_pool(name="l_sb", bufs=3))
        r_sbp = ctx.enter_context(tc.tile_pool(name="r_sb", bufs=3))
        y_sbp = ctx.enter_context(tc.tile_pool(name="y_sb", bufs=2))

        r2t = cst_pool.tile([128, 128], BF16)
        nc.sync.dma_start(r2t[:], r2t_d)
        wv_sb = cst_pool.tile([128, CC, HPC * D], BF16)
        wo_sb = cst_pool.tile([128, NP, C], BF16)
        utri = cst_pool.tile([128, 128], BF16)
        eband = cst_pool.tile([128, 128], BF16)
        exp_warm = cst_pool.tile([1, 2], F32)

        nc.gpsimd.memset(v_aug[:, :, :, D:D + 1], 1.0)

        # ---------- attention building blocks ----------
        def emit_scores_offdiag(p, qc, kcs):
            s_t = [s_psp.tile([128, 1024], F32, tag=f"s{h01}",
                              name=f"s_t{h01}") for h01 in (0, 1)]
            for j, kc in enumerate(kcs):
                for h01 in (0, 1):
                    r0, r1 = h01 * 64, h01 * 64 + 64
                    nc.tensor.matmul(
                        s_t[h01][:, j * 512:(j + 1) * 512],
                        k_all[r0:r1, p, kc * 128:(kc + 1) * 128],
                        q_all[r0:r1, p, qc * 512:(qc + 1) * 512],
                        start=True, stop=True,
                    )
            pts = []
            for h01 in (0, 1):
                pt = p_sbp.tile([128, 1024], BF16, tag=f"pt{h01}")
                w = len(kcs) * 512
                nc.scalar.activation(pt[:, 0:w], s_t[h01][:, 0:w], EXP,
                                     scale=0.125)
                pts.append(pt)
            return pts

        def emit_pv_offdiag(p, qc, kcs, pts, pv):
            for j, kc in enumerate(kcs):
                for h01 in (0, 1):
                    nc.tensor.matmul(
                        pv[h01][:],
                        v_aug[:, kc, p * 2 + h01, :],
                        pts[h01][:, j * 512:(j + 1) * 512],
                        start=(kc == 0), stop=False,
                    )

        # diagonal tiles: half 0 = j0(512)+j1(384), half 1 = j2(256)+j3(128)
        DIAG_SEGS = (((0, 0, 512), (1, 512, 384)),
                     ((2, 0, 256), (3, 256, 128)))

        def emit_scores_diag(p, qc, segs):
            s_d = [s_psp.tile([128, 1024], F32, tag=f"s{h01}",
                              name=f"s_d{h01}") for h01 in (0, 1)]
            for j, off, wj in segs:
                kc = 4 * qc + j
                for h01 in (0, 1):
                    r0, r1 = h01 * 64, h01 * 64 + 64
                    nc.tensor.matmul(
                        s_d[h01][:, off:off + wj],
                        k_all[r0:r1, p, kc * 128:(kc + 1) * 128],
                        q_all[r0:r1, p, qc * 512 + 128 * j:qc * 512 + 512],
                        start=True, stop=True,
                    )
            pts = []
            for h01 in (0, 1):
                pt_d = p_sbp.tile([128, 1024], BF16, tag=f"pt{h01}",
                                  name="pt_d")
                wtot = sum(sg[2] for sg in segs)
                nc.scalar.activation(pt_d[:, 0:wtot], s_d[h01][:, 0:wtot],
                                     EXP, scale=0.125)
                # causal keep-mask (utri[k,q] = k<=q) on each seg's leading
                # 128 cols, on DVE instead of utri@eband matmul adds on PE
                for j, off, wj in segs:
                    nc.vector.tensor_mul(pt_d[:, off:off + 128],
                                         pt_d[:, off:off + 128], utri[:])
                pts.append(pt_d)
            return pts

        def emit_pv_diag(p, qc, segs, pts, pv, last):
            for j, off, wj in segs:
                kc = 4 * qc + j
                for h01 in (0, 1):
                    nc.tensor.matmul(
                        pv[h01][:, 128 * j:512],
                        v_aug[:, kc, p * 2 + h01, :],
                        pts[h01][:, off:off + wj],
                        start=(kc == 0), stop=(last and j == 3),
                    )

        def emit_norm(p, qc, pv, tail=False):
            for h01 in (0, 1):
                lrow = l_sbp.tile([1, 512], F32, tag=f"l{h01}")
                if tail:
                    nc.scalar.copy(lrow[0:1, :], pv[h01][64:65, :])
                else:
                    nc.vector.tensor_copy(lrow[0:1, :], pv[h01][64:65, :])
                rbc = r_sbp.tile([64, 512], F32, tag=f"r{h01}")
                nc.gpsimd.partition_broadcast(rbc[:], lrow[0:1, :],
                                              channels=64)
                nc.vector.reciprocal_approx_fast(rbc[:], rbc[:])
                nc.vector.tensor_mul(
                    out_norm[h01 * 64:h01 * 64 + 64, p,
                             qc * 512:(qc + 1) * 512],
                    pv[h01][0:64, :],
                    rbc[:],
                )

        def attn_units(p, qc):
            units = []
            for g0 in range(0, 4 * qc, 2):
                kcs = list(range(g0, min(g0 + 2, 4 * qc)))
                units.append((
                    (lambda kk: lambda: emit_scores_offdiag(p, qc, kk))(kcs),
                    (lambda kk: lambda pts, pv: emit_pv_offdiag(
                        p, qc, kk, pts, pv))(kcs),
                ))
            for half, segs in enumerate(DIAG_SEGS):
                units.append((
                    (lambda ss: lambda: emit_scores_diag(p, qc, ss))(segs),
                    (lambda ss, la: lambda pts, pv: emit_pv_diag(
                        p, qc, ss, pts, pv, last=la))(segs, half == 1),
                ))
            return units

        # ---- phase 1 + leading (p0, qc3) scores ----
        with tc.tile_pool(name="w", bufs=1) as w_pool, \
             tc.tile_pool(name="const", bufs=1) as const_pool, \
             tc.tile_pool(name="p1ps", bufs=4, space="PSUM") as p1ps, \
             tc.tile_pool(name="p1tmp", bufs=2) as p1tmp:
            wq_sb = w_pool.tile([128, CC, HPC * D], BF16)
            nc.sync.dma_start(wq_sb[:], wq_d.rearrange("(cc p) d -> p cc d", p=128))
            wk_sb = w_pool.tile([128, CC, HPC * D], BF16)
            ccat = const_pool.tile([128, T], BF16)
            scat = const_pool.tile([128, T], BF16)
            nc.scalar.dma_start(wk_sb[:], wk_d.rearrange("(cc p) d -> p cc d", p=128))
            xT_r = xT_d.rearrange("(cc p) t -> p cc t", p=128)
            x_engs = (nc.sync, nc.scalar, nc.gpsimd, nc.sync, nc.scalar,
                      nc.gpsimd)
            for cc in range(CC):
                x_engs[cc].dma_start(xt_sb[:, cc, :], xT_r[:, cc, :])
            nc.sync.dma_start(ccat[:], ccat_d)
            nc.scalar.dma_start(scat[:], scat_d)
            nc.sync.dma_start(utri[:], utri_d)
            nc.scalar.dma_start(eband[:], eband_d)
            nc.gpsimd.dma_start(wv_sb[:], wv_d.rearrange("(cc p) d -> p cc d", p=128))
            nc.gpsimd.dma_start(wo_sb[:], wo_d.rearrange("(hc p) c -> p hc c", p=128))

            # HAM warmup + Exp table preload while the input DMAs land
            warm_t = p1ps.tile([128, 1024], F32, tag="p1")
            warm = warm_t[:, 0:128]
            nc.scalar.activation(exp_warm[:], r2t[0:1, 0:2], EXP)
            for _ in range(44):
                nc.tensor.matmul(warm[:], r2t[:], r2t[:], start=True, stop=True)

            def finish_block(blk):
                dt, dst, qraw, sin_t = blk
                for hh in range(2):
                    hsl = slice(hh * 1024, (hh + 1) * 1024)
                    ps_r = p1ps.tile([128, 1024], F32, tag="p1", name="ps_r")
                    for tq in range(2):
                        nc.tensor.matmul(
                            ps_r[:, tq * 512:(tq + 1) * 512],
                            r2t[:],
                            qraw[:, hh * 1024 + tq * 512:
                                  hh * 1024 + (tq + 1) * 512],
                            start=True, stop=True,
                        )
                    nc.vector.tensor_mul(sin_t[:, hsl], ps_r[:, :], scat[:, hsl])
                nc.vector.tensor_mul(dst[:, dt, :], qraw[:], ccat[:])
                nc.vector.tensor_add(dst[:, dt, :], dst[:, dt, :], sin_t[:])

            blocks = [(dt, w_sb, dst)
                      for dt in range(NP)
                      for w_sb, dst in ((wq_sb, q_all), (wk_sb, k_all))]

            # The first two blocks run cc-interleaved: four PSUM chains
            # consume each x chunk as its DMA lands (the load is HBM-bound,
            # ~3us/chunk), instead of one chain starving on later chunks.
            lead, lead_ps = [], []
            for dt, w_sb, dst in blocks[:2]:
                qraw = p1tmp.tile([128, T], BF16, tag="qraw")
                sin_t = p1tmp.tile([128, T], BF16, tag="sin")
                ps_pair = [p1ps.tile([128, 1024], F32, tag="p1", name="ps_q")
                           for _ in range(2)]
                lead.append((dt, w_sb, dst, qraw, sin_t))
                lead_ps.append(ps_pair)
            for cc in range(CC):
                for bi, (dt, w_sb, dst, qraw, sin_t) in enumerate(lead):
                    for hh in range(2):
                        for tq in range(2):
                            nc.tensor.matmul(
                                lead_ps[bi][hh][:, tq * 512:(tq + 1) * 512],
                                w_sb[:, cc, dt * 128:(dt + 1) * 128],
                                xt_sb[:, cc,
                                      hh * 1024 + tq * 512:
                                      hh * 1024 + (tq + 1) * 512],
                                start=(cc == 0), stop=(cc == CC - 1),
                            )
            for bi, (dt, w_sb, dst, qraw, sin_t) in enumerate(lead):
                for hh in range(2):
                    hsl = slice(hh * 1024, (hh + 1) * 1024)
                    nc.scalar.copy(qraw[:, hsl], lead_ps[bi][hh][:, :])

            prev = (lead[1][0], lead[1][2], lead[1][3], lead[1][4])
            finish_block((lead[0][0], lead[0][2], lead[0][3], lead[0][4]))
            for dt, w_sb, dst in blocks[2:]:
                qraw = p1tmp.tile([128, T], BF16, tag="qraw")
                sin_t = p1tmp.tile([128, T], BF16, tag="sin")
                for hh in range(2):
                    hsl = slice(hh * 1024, (hh + 1) * 1024)
                    ps_q = p1ps.tile([128, 1024], F32, tag="p1", name="ps_q")
                    for cc in range(CC):
                        for tq in range(2):
                            nc.tensor.matmul(
                                ps_q[:, tq * 512:(tq + 1) * 512],
                                w_sb[:, cc, dt * 128:(dt + 1) * 128],
                                xt_sb[:, cc,
                                      hh * 1024 + tq * 512:
                                      hh * 1024 + (tq + 1) * 512],
                                start=(cc == 0), stop=(cc == CC - 1),
                            )
                    nc.scalar.copy(qraw[:, hsl], ps_q[:, :])
                finish_block(prev)
                prev = (dt, dst, qraw, sin_t)

            finish_block(prev)

        # ---- phase 2: attention; vproj zipped into (p0, qc3) ----
        with tc.tile_pool(name="s_ps", bufs=1, space="PSUM") as s_psp, \
             tc.tile_pool(name="aux_ps", bufs=4, space="PSUM") as aux_psp:

            def emit_vproj(tt):
                ps_v = aux_psp.tile([128, HPC * D], F32, tag="aux", name="ps_v")
                for cc in range(CC):
                    nc.tensor.matmul(
                        ps_v[:, 0:HPC * D],
                        xt_sb[:, cc, tt * 128:(tt + 1) * 128],
                        wv_sb[:, cc, :],
                        start=(cc == 0), stop=(cc == CC - 1),
                    )
                nc.vector.tensor_copy(
                    v_aug[:, tt, :, 0:D],
                    ps_v[:, 0:HPC * D].rearrange("p (h d) -> p h d", d=D),
                )

            def emit_outproj(qc, tail=False):
                for tt in range(4 * qc, 4 * qc + 4):
                    y_a = aux_psp.tile([128, 512], F32, tag="aux", name="y_a")
                    y_b = aux_psp.tile([128, 256], F32, tag="aux", name="y_b")
                    for hc in range(NP):
                        lhsT = out_norm[:, hc, tt * 128:(tt + 1) * 128]
                        nc.tensor.matmul(y_a[:, 0:512], lhsT,
                                         wo_sb[:, hc, 0:512],
                                         start=(hc == 0), stop=(hc == NP - 1))
                        nc.tensor.matmul(y_b[:, 0:256], lhsT,
                                         wo_sb[:, hc, 512:768],
                                         start=(hc == 0), stop=(hc == NP - 1))
                    yt = y_sbp.tile([128, C], F32, tag="yt")
                    if tail:
                        nc.scalar.copy(yt[:, 0:512], y_a[:, 0:512])
                        nc.scalar.copy(yt[:, 512:768], y_b[:, 0:256])
                    else:
                        nc.vector.tensor_copy(yt[:, 0:512], y_a[:, 0:512])
                        nc.vector.tensor_copy(yt[:, 512:768], y_b[:, 0:256])
                    nc.sync.dma_start(y_d[tt * 128:(tt + 1) * 128, :], yt[:])

            def emit_attn(p, qc, pv, fillers=None, units=None, pre=None):
                """Ping-pong with 1-group score lookahead; optional PE filler
                work (e.g. vproj closures) interleaved between groups."""
                fillers = list(fillers or [])
                fi = 0
                queue = list(pre or [])   # [(pv_fn, pts)] already scored
                for si, (sc_fn, pv_fn) in enumerate(units if units is not None
                                                    else attn_units(p, qc)):
                    queue.append((pv_fn, sc_fn()))
                    if len(queue) > 1:
                        fn, pts = queue.pop(0)
                        fn(pts, pv)
                    while fi < len(fillers) and fi < (si + 1) * 2:
                        fillers[fi]()
                        fi += 1
                while fi < len(fillers):
                    fillers[fi]()
                    fi += 1
                for fn, pts in queue:
                    fn(pts, pv)
                emit_norm(p, qc, pv)

            # qc=3: p0 continues from the phase-1 prefetched groups, with
            # the v-projection zipped in as PE filler
            vproj_fillers = [(lambda t: lambda: emit_vproj(t))(tt)
                             for tt in range(KC)]
            pv = [aux_psp.tile([65, 512], F32, tag="aux",
                               name=f"pv{h01}") for h01 in (0, 1)]
            emit_attn(0, 3, pv, fillers=vproj_fillers)
            for p in (1, 2):
                pv = [aux_psp.tile([65, 512], F32, tag="aux",
                                   name=f"pv{h01}") for h01 in (0, 1)]
                emit_attn(p, 3, pv)
            emit_outproj(3)
            for p in range(NP):
                pv = [aux_psp.tile([65, 512], F32, tag="aux",
                                   name=f"pv{h01}") for h01 in (0, 1)]
                emit_attn(p, 2, pv)
            emit_outproj(2)

            # qc=1 and qc=0 tails: breadth-first across head-pairs to
            # overlap the short scores/exp/PV/norm latency chains; qc=0's
            # scores/exps run under qc=1's PV phase and outproj(1) fills
            # the PE during qc=0's exps
            helds1 = [[(pv_fn, sc_fn()) for sc_fn, pv_fn in attn_units(p, 1)]
                      for p in range(NP)]
            for p in range(NP):
                pv = [aux_psp.tile([65, 512], F32, tag="aux",
                                   name=f"pv{h01}") for h01 in (0, 1)]
                for pv_fn, pts in helds1[p]:
                    pv_fn(pts, pv)
                emit_norm(p, 1, pv, tail=True)
            helds0 = [[(pv_fn, sc_fn()) for sc_fn, pv_fn in attn_units(p, 0)]
                      for p in range(NP)]
            emit_outproj(1)
            for p in range(NP):
                pv = [aux_psp.tile([65, 512], F32, tag="aux",
                                   name=f"pv{h01}") for h01 in (0, 1)]
                for pv_fn, pts in helds0[p]:
                    pv_fn(pts, pv)
                emit_norm(p, 0, pv, tail=True)
            emit_outproj(0, tail=True)

    nc.compile()
    return nc


# make mybir importable inside _build_program's nested scopes
from concourse import mybir  # noqa: E402


def _get_compiled():
    global _COMPILED
    if _COMPILED is None:
        _COMPILED = _build_program()
    return _COMPILED


def _make_in_maps(inputs):
    import ml_dtypes

    BF = ml_dtypes.bfloat16
    x = np.asarray(inputs["x"], dtype=np.float32)
    wq = np.asarray(inputs["wq"], dtype=np.float32).astype(BF)
    wk = np.asarray(inputs["wk"], dtype=np.float32).astype(BF)
    wv = np.asarray(inputs["wv"], dtype=np.float32).astype(BF)
    wo = np.asarray(inputs["wo"], dtype=np.float32).astype(BF)

    ccat, scat = _rope_tables()
    r2t = _rot_matrix()
    m = np.arange(128)
    utri = (m[:, None] <= m[None, :]).astype(BF)
    eband = np.zeros((128, 128), dtype=np.float32)
    eband[np.arange(1, 128), np.arange(127)] = -1e9
    eband = eband.astype(BF)

    xTs = [np.ascontiguousarray(x[b].T.astype(BF)) for b in range(B)]
    in_maps = []
    for c in range(8):
        b, g = c // 2, c % 2
        sl = slice(g * HPC * D, (g + 1) * HPC * D)
        in_maps.append(dict(
            xT=xTs[b],
            wq=np.ascontiguousarray(wq[:, sl]),
            wk=np.ascontiguousarray(wk[:, sl]),
            wv=np.ascontiguousarray(wv[:, sl]),
            wo=np.ascontiguousarray(wo[sl, :]),
            ccat=ccat, scat=scat, r2t=r2t, utri=utri, eband=eband,
        ))
    return in_maps


def kernel(x, wq, wk, wv, wo, mask):
    """Full inputs in, full output out. Shards across 8 NeuronCores internally.

    The mask input is the standard causal mask produced by setup_inputs();
    causality is implemented directly on-device.
    """
    from concourse.bass_utils import run_bass_kernel_spmd

    in_maps = _make_in_maps(dict(x=x, wq=wq, wk=wk, wv=wv, wo=wo))

    nc = _get_compiled()
    res = run_bass_kernel_spmd(nc, in_maps, list(range(8)))
    out = np.empty((B, T, C), dtype=np.float32)
    for b in range(B):
        out[b] = res.results[2 * b]["y"] + res.results[2 * b + 1]["y"]
    return out



# revision 1
# speedup vs baseline: 1.6281x; 1.6281x over previous
"""Causal multi-head attention with RoPE for Trainium2, sharded over 8 NeuronCores.

Problem: B=4, T=2048, C=768, H=12, D=64, fp32 in/out.
    q,k,v = x @ wq/wk/wv  (per-head reshape), RoPE(q,k), causal softmax(q k^T/sqrt(D)) v,
    out = concat_heads @ wo.

Sharding: core c -> (batch b = c//2, head-group g = c%2 covering heads g*6..g*6+5).
Each core computes its 6 heads' attention and a partial output projection
y_c = out_heads(g) @ wo[rows g]; the host sums the two partials per batch.

On-core dataflow (bf16 matmul operands, fp32 PSUM accumulation; bf16 enables
Fast Weight Load and 1024-wide moving operands, so the q/k projection and
rotation chains use half the matmul instructions):
  - host passes x^T (bf16); input DMAs split across all three DGE queues,
    critical tensors first; a HAM-warmup matmul stream covers the load.
  - q^T,k^T in [head_dim, T] layout; RoPE via a block-rotation matmul +
    cos/sin tensor ops on DVE, pipelined one block behind the projection
    so the PSUM->SBUF copy latency never stalls the PE.
  - scores transposed: S^T[k, q] = k^T.T @ q^T with K=64 row-pairing
    (even head at partitions 0:64, odd at 64:128 -> concurrent row groups).
    The first two (p=0, qc=3) score groups are emitted inside the phase-1
    tail and the rest zip against the v-projection, so the ScalarE exp
    stream (the second serial resource, ~13.4M exps/core) starts the
    moment phase 1 drains.
  - P = exp(S/8) on ScalarE -> bf16; causal masking of diagonal tiles via
    a bf16 utri@eband matmul add before exp.
  - PV with a ones-row appended to V: out_unnorm^T[d, q] and l[q] in one
    accumulated matmul chain per (head, q-chunk); 1-group score lookahead
    keeps the PE ahead of the exp stream.
  - normalization: l row -> partition 0, gpsimd.partition_broadcast,
    reciprocal_approx_fast, TT multiply.  The small qc=1/qc=0 tails run
    breadth-first across head-pairs with their PSUM->SBUF copies moved to
    the by-then idle ScalarE.
  - output projection accumulates 3 head-pair chunks into [128, 768] PSUM.
"""

import numpy as np
from contextlib import ExitStack

B, T, C, H, D = 4, 2048, 768, 12, 64
HPC = 6          # heads per core
NP = 3           # head-pair tiles per core
CC = C // 128    # 6 contraction chunks
TT = T // 128    # 16 t tiles
QC = T // 512    # 4 q chunks
KC = T // 128    # 16 k chunks

_COMPILED = None


def _rope_tables():
    import ml_dtypes
    inv_freq = 1.0 / (10000.0 ** (np.arange(0, D, 2, dtype=np.float64) / D))  # [32]
    t = np.arange(T, dtype=np.float64)
    freqs = np.outer(t, inv_freq)                      # [T, 32]
    cosT = np.cos(freqs).T.astype(np.float32)          # [32, T]
    sinT = np.sin(freqs).T.astype(np.float32)
    ccat = np.tile(cosT, (4, 1)).astype(ml_dtypes.bfloat16)   # [128, T]
    scat = np.tile(sinT, (4, 1)).astype(ml_dtypes.bfloat16)
    return np.ascontiguousarray(ccat), np.ascontiguousarray(scat)


def _rot_matrix():
    import ml_dtypes
    # rotate_half as a matmul: rot = R @ q (q in [D, T] layout), per 64-row block
    R = np.zeros((D, D), dtype=np.float32)
    R[0:32, 32:64] = -np.eye(32, dtype=np.float32)
    R[32:64, 0:32] = np.eye(32, dtype=np.float32)
    R2 = np.zeros((128, 128), dtype=np.float32)
    R2[0:64, 0:64] = R
    R2[64:128, 64:128] = R
    return np.ascontiguousarray(R2.T.astype(ml_dtypes.bfloat16))  # lhsT for out = R2 @ q


def _build_program():
    import concourse.tile as tile
    from concourse import bacc, mybir

    F32 = mybir.dt.float32
    BF16 = mybir.dt.bfloat16
    EXP = mybir.ActivationFunctionType.Exp

    nc = bacc.Bacc("TRN2", target_bir_lowering=False, debug=False, num_devices=8)

    xT_d = nc.dram_tensor("xT", [C, T], BF16, kind="ExternalInput").ap()
    wq_d = nc.dram_tensor("wq", [C, HPC * D], BF16, kind="ExternalInput").ap()
    wk_d = nc.dram_tensor("wk", [C, HPC * D], BF16, kind="ExternalInput").ap()
    wv_d = nc.dram_tensor("wv", [C, HPC * D], BF16, kind="ExternalInput").ap()
    wo_d = nc.dram_tensor("wo", [HPC * D, C], BF16, kind="ExternalInput").ap()
    ccat_d = nc.dram_tensor("ccat", [128, T], BF16, kind="ExternalInput").ap()
    scat_d = nc.dram_tensor("scat", [128, T], BF16, kind="ExternalInput").ap()
    r2t_d = nc.dram_tensor("r2t", [128, 128], BF16, kind="ExternalInput").ap()
    utri_d = nc.dram_tensor("utri", [128, 128], BF16, kind="ExternalInput").ap()
    eband_d = nc.dram_tensor("eband", [128, 128], BF16, kind="ExternalInput").ap()
    y_d = nc.dram_tensor("y", [T, C], F32, kind="ExternalOutput").ap()

    with tile.TileContext(nc) as tc, ExitStack() as ctx:
        big_pool = ctx.enter_context(tc.tile_pool(name="big", bufs=1))
        q_all = big_pool.tile([128, NP, T], BF16)
        k_all = big_pool.tile([128, NP, T], BF16)
        v_aug = big_pool.tile([128, KC, HPC, D + 1], BF16)
        out_norm = big_pool.tile([128, NP, T], BF16)

        cst_pool = ctx.enter_context(tc.tile_pool(name="cst", bufs=1))
        xt_pool = ctx.enter_context(tc.tile_pool(name="xt", bufs=1))
        xt_sb = xt_pool.tile([128, CC, T], BF16)

        p_sbp = ctx.enter_context(tc.tile_pool(name="p_sb", bufs=12))
        l_sbp = ctx.enter_context(tc.tile_pool(name="l_sb", bufs=3))
        r_sbp = ctx.enter_context(tc.tile_pool(name="r_sb", bufs=3))
        y_sbp = ctx.enter_context(tc.tile_pool(name="y_sb", bufs=2))

        r2t = cst_pool.tile([128, 128], BF16)
        nc.sync.dma_start(r2t[:], r2t_d)
        wv_sb = cst_pool.tile([128, CC, HPC * D], BF16)
        wo_sb = cst_pool.tile([128, NP, C], BF16)
        utri = cst_pool.tile([128, 128], BF16)
        eband = cst_pool.tile([128, 128], BF16)
        exp_warm = cst_pool.tile([1, 2], F32)

        nc.gpsimd.memset(v_aug[:, :, :, D:D + 1], 1.0)

        # ---------- attention building blocks ----------
        def emit_scores_offdiag(p, qc, kcs):
            s_t = [s_psp.tile([128, 1024], F32, tag=f"s{h01}",
                              name=f"s_t{h01}") for h01 in (0, 1)]
            for j, kc in enumerate(kcs):
                for h01 in (0, 1):
                    r0, r1 = h01 * 64, h01 * 64 + 64
                    nc.tensor.matmul(
                        s_t[h01][:, j * 512:(j + 1) * 512],
                        k_all[r0:r1, p, kc * 128:(kc + 1) * 128],
                        q_all[r0:r1, p, qc * 512:(qc + 1) * 512],
                        start=True, stop=True,
                    )
            pts = []
            for h01 in (0, 1):
                pt = p_sbp.tile([128, 1024], BF16, tag=f"pt{h01}")
                w = len(kcs) * 512
                nc.scalar.activation(pt[:, 0:w], s_t[h01][:, 0:w], EXP,
                                     scale=0.125)
                pts.append(pt)
            return pts

        def emit_pv_offdiag(p, qc, kcs, pts, pv):
            for j, kc in enumerate(kcs):
                for h01 in (0, 1):
                    nc.tensor.matmul(
                        pv[h01][:],
                        v_aug[:, kc, p * 2 + h01, :],
                        pts[h01][:, j * 512:(j + 1) * 512],
                        start=(kc == 0), stop=False,
                    )

        # diagonal tiles: half 0 = j0(512)+j1(384), half 1 = j2(256)+j3(128)
        DIAG_SEGS = (((0, 0, 512), (1, 512, 384)),
                     ((2, 0, 256), (3, 256, 128)))

        def emit_scores_diag(p, qc, segs):
            s_d = [s_psp.tile([128, 1024], F32, tag=f"s{h01}",
                              name=f"s_d{h01}") for h01 in (0, 1)]
            for j, off, wj in segs:
                kc = 4 * qc + j
                for h01 in (0, 1):
                    r0, r1 = h01 * 64, h01 * 64 + 64
                    nc.tensor.matmul(
                        s_d[h01][:, off:off + wj],
                        k_all[r0:r1, p, kc * 128:(kc + 1) * 128],
                        q_all[r0:r1, p, qc * 512 + 128 * j:qc * 512 + 512],
                        start=True, stop=True,
                    )
            pts = []
            for h01 in (0, 1):
                pt_d = p_sbp.tile([128, 1024], BF16, tag=f"pt{h01}",
                                  name="pt_d")
                wtot = sum(sg[2] for sg in segs)
                nc.scalar.activation(pt_d[:, 0:wtot], s_d[h01][:, 0:wtot],
                                     EXP, scale=0.125)
                # causal keep-mask (utri[k,q] = k<=q) on each seg's leading
                # 128 cols, on DVE instead of utri@eband matmul adds on PE
                for j, off, wj in segs:
                    nc.vector.tensor_mul(pt_d[:, off:off + 128],
                                         pt_d[:, off:off + 128], utri[:])
                pts.append(pt_d)
            return pts

        def emit_pv_diag(p, qc, segs, pts, pv, last):
            for j, off, wj in segs:
                kc = 4 * qc + j
                for h01 in (0, 1):
                    nc.tensor.matmul(
                        pv[h01][:, 128 * j:512],
                        v_aug[:, kc, p * 2 + h01, :],
                        pts[h01][:, off:off + wj],
                        start=(kc == 0), stop=(last and j == 3),
                    )

        def emit_norm(p, qc, pv, tail=False):
            for h01 in (0, 1):
                lrow = l_sbp.tile([1, 512], F32, tag=f"l{h01}")
                if tail:
                    nc.scalar.copy(lrow[0:1, :], pv[h01][64:65, :])
                else:
                    nc.vector.tensor_copy(lrow[0:1, :], pv[h01][64:65, :])
                rbc = r_sbp.tile([64, 512], F32, tag=f"r{h01}")
                nc.gpsimd.partition_broadcast(rbc[:], lrow[0:1, :],
                                              channels=64)
                nc.vector.reciprocal_approx_fast(rbc[:], rbc[:])
                nc.vector.tensor_mul(
                    out_norm[h01 * 64:h01 * 64 + 64, p,
                             qc * 512:(qc + 1) * 512],
                    pv[h01][0:64, :],
                    rbc[:],
                )

        def attn_units(p, qc):
            units = []
            for g0 in range(0, 4 * qc, 2):
                kcs = list(range(g0, min(g0 + 2, 4 * qc)))
                units.append((
                    (lambda kk: lambda: emit_scores_offdiag(p, qc, kk))(kcs),
                    (lambda kk: lambda pts, pv: emit_pv_offdiag(
                        p, qc, kk, pts, pv))(kcs),
                ))
            for half, segs in enumerate(DIAG_SEGS):
                units.append((
                    (lambda ss: lambda: emit_scores_diag(p, qc, ss))(segs),
                    (lambda ss, la: lambda pts, pv: emit_pv_diag(
                        p, qc, ss, pts, pv, last=la))(segs, half == 1),
                ))
            return units

        # ---- phase 1 + leading (p0, qc3) scores ----
        with tc.tile_pool(name="w", bufs=1) as w_pool, \
             tc.tile_pool(name="const", bufs=1) as const_pool, \
             tc.tile_pool(name="p1ps", bufs=4, space="PSUM") as p1ps, \
             tc.tile_pool(name="p1tmp", bufs=2) as p1tmp:
            wq_sb = w_pool.tile([128, CC, HPC * D], BF16)
            nc.sync.dma_start(wq_sb[:], wq_d.rearrange("(cc p) d -> p cc d", p=128))
            wk_sb = w_pool.tile([128, CC, HPC * D], BF16)
            ccat = const_pool.tile([128, T], BF16)
            scat = const_pool.tile([128, T], BF16)
            nc.scalar.dma_start(wk_sb[:], wk_d.rearrange("(cc p) d -> p cc d", p=128))
            xT_r = xT_d.rearrange("(cc p) t -> p cc t", p=128)
            x_engs = (nc.sync, nc.scalar, nc.gpsimd, nc.sync, nc.scalar,
                      nc.gpsimd)
            for cc in range(CC):
                x_engs[cc].dma_start(xt_sb[:, cc, :], xT_r[:, cc, :])
            nc.sync.dma_start(ccat[:], ccat_d)
            nc.scalar.dma_start(scat[:], scat_d)
            nc.sync.dma_start(utri[:], utri_d)
            nc.scalar.dma_start(eband[:], eband_d)
            nc.gpsimd.dma_start(wv_sb[:], wv_d.rearrange("(cc p) d -> p cc d", p=128))
            nc.gpsimd.dma_start(wo_sb[:], wo_d.rearrange("(hc p) c -> p hc c", p=128))

            # HAM warmup + Exp table preload while the input DMAs land
            warm_t = p1ps.tile([128, 1024], F32, tag="p1")
            warm = warm_t[:, 0:128]
            nc.scalar.activation(exp_warm[:], r2t[0:1, 0:2], EXP)
            for _ in range(44):
                nc.tensor.matmul(warm[:], r2t[:], r2t[:], start=True, stop=True)

            def finish_block(blk):
                dt, dst, qraw, sin_t = blk
                for hh in range(2):
                    hsl = slice(hh * 1024, (hh + 1) * 1024)
                    ps_r = p1ps.tile([128, 1024], F32, tag="p1", name="ps_r")
                    for tq in range(2):
                        nc.tensor.matmul(
                            ps_r[:, tq * 512:(tq + 1) * 512],
                            r2t[:],
                            qraw[:, hh * 1024 + tq * 512:
                                  hh * 1024 + (tq + 1) * 512],
                            start=True, stop=True,
                        )
                    nc.vector.tensor_mul(sin_t[:, hsl], ps_r[:, :], scat[:, hsl])
                nc.vector.tensor_mul(dst[:, dt, :], qraw[:], ccat[:])
                nc.vector.tensor_add(dst[:, dt, :], dst[:, dt, :], sin_t[:])

            blocks = [(dt, w_sb, dst)
                      for dt in range(NP)
                      for w_sb, dst in ((wq_sb, q_all), (wk_sb, k_all))]

            # The first two blocks run cc-interleaved: four PSUM chains
            # consume each x chunk as its DMA lands (the load is HBM-bound,
            # ~3us/chunk), instead of one chain starving on later chunks.
            lead, lead_ps = [], []
            for dt, w_sb, dst in blocks[:2]:
                qraw = p1tmp.tile([128, T], BF16, tag="qraw")
                sin_t = p1tmp.tile([128, T], BF16, tag="sin")
                ps_pair = [p1ps.tile([128, 1024], F32, tag="p1", name="ps_q")
                           for _ in range(2)]
                lead.append((dt, w_sb, dst, qraw, sin_t))
                lead_ps.append(ps_pair)
            for cc in range(CC):
                for bi, (dt, w_sb, dst, qraw, sin_t) in enumerate(lead):
                    for hh in range(2):
                        for tq in range(2):
                            nc.tensor.matmul(
                                lead_ps[bi][hh][:, tq * 512:(tq + 1) * 512],
                                w_sb[:, cc, dt * 128:(dt + 1) * 128],
                                xt_sb[:, cc,
                                      hh * 1024 + tq * 512:
                                      hh * 1024 + (tq + 1) * 512],
                                start=(cc == 0), stop=(cc == CC - 1),
                            )
            for bi, (dt, w_sb, dst, qraw, sin_t) in enumerate(lead):
                for hh in range(2):
                    hsl = slice(hh * 1024, (hh + 1) * 1024)
                    nc.scalar.copy(qraw[:, hsl], lead_ps[bi][hh][:, :])

            prev = (lead[1][0], lead[1][2], lead[1][3], lead[1][4])
            finish_block((lead[0][0], lead[0][2], lead[0][3], lead[0][4]))
            for dt, w_sb, dst in blocks[2:]:
                qraw = p1tmp.tile([128, T], BF16, tag="qraw")
                sin_t = p1tmp.tile([128, T], BF16, tag="sin")
                for hh in range(2):
                    hsl = slice(hh * 1024, (hh + 1) * 1024)
                    ps_q = p1ps.tile([128, 1024], F32, tag="p1", name="ps_q")
                    for cc in range(CC):
                        for tq in range(2):
                            nc.tensor.matmul(
                                ps_q[:, tq * 512:(tq + 1) * 512],
                                w_sb[:, cc, dt * 128:(dt + 1) * 128],
                                xt_sb[:, cc,
                                      hh * 1024 + tq * 512:
                                      hh * 1024 + (tq + 1) * 512],
                                start=(cc == 0), stop=(cc == CC - 1),
                            )
                    nc.scalar.copy(qraw[:, hsl], ps_q[:, :])
                finish_block(prev)
                prev = (dt, dst, qraw, sin_t)

            finish_block(prev)

        # ---- phase 2: attention; vproj zipped into (p0, qc3) ----
        with tc.tile_pool(name="s_ps", bufs=1, space="PSUM") as s_psp, \
             tc.tile_pool(name="aux_ps", bufs=4, space="PSUM") as aux_psp:

            def emit_vproj(tt):
                ps_v = aux_psp.tile([128, HPC * D], F32, tag="aux", name="ps_v")
                for cc in range(CC):
                    nc.tensor.matmul(
                        ps_v[:, 0:HPC * D],
                        xt_sb[:, cc, tt * 128:(tt + 1) * 128],
                        wv_sb[:, cc, :],
                        start=(cc == 0), stop=(cc == CC - 1),
                    )
                nc.vector.tensor_copy(
                    v_aug[:, tt, :, 0:D],
                    ps_v[:, 0:HPC * D].rearrange("p (h d) -> p h d", d=D),
                )

            def emit_outproj(qc, tail=False):
                for tt in range(4 * qc, 4 * qc + 4):
                    y_a = aux_psp.tile([128, 512], F32, tag="aux", name="y_a")
                    y_b = aux_psp.tile([128, 256], F32, tag="aux", name="y_b")
                    for hc in range(NP):
                        lhsT = out_norm[:, hc, tt * 128:(tt + 1) * 128]
                        nc.tensor.matmul(y_a[:, 0:512], lhsT,
                                         wo_sb[:, hc, 0:512],
                                         start=(hc == 0), stop=(hc == NP - 1))
                        nc.tensor.matmul(y_b[:, 0:256], lhsT,
                                         wo_sb[:, hc, 512:768],
                                         start=(hc == 0), stop=(hc == NP - 1))
                    yt = y_sbp.tile([128, C], F32, tag="yt")
                    if tail:
                        nc.scalar.copy(yt[:, 0:512], y_a[:, 0:512])
                        nc.scalar.copy(yt[:, 512:768], y_b[:, 0:256])
                    else:
                        nc.vector.tensor_copy(yt[:, 0:512], y_a[:, 0:512])
                        nc.vector.tensor_copy(yt[:, 512:768], y_b[:, 0:256])
                    nc.sync.dma_start(y_d[tt * 128:(tt + 1) * 128, :], yt[:])

            def emit_attn(p, qc, pv, fillers=None, units=None, pre=None):
                """Ping-pong with 1-group score lookahead; optional PE filler
                work (e.g. vproj closures) interleaved between groups."""
                fillers = list(fillers or [])
                fi = 0
                queue = list(pre or [])   # [(pv_fn, pts)] already scored
                for si, (sc_fn, pv_fn) in enumerate(units if units is not None
                                                    else attn_units(p, qc)):
                    queue.append((pv_fn, sc_fn()))
                    if len(queue) > 1:
                        fn, pts = queue.pop(0)
                        fn(pts, pv)
                    while fi < len(fillers) and fi < (si + 1) * 2:
                        fillers[fi]()
                        fi += 1
                while fi < len(fillers):
                    fillers[fi]()
                    fi += 1
                for fn, pts in queue:
                    fn(pts, pv)
                emit_norm(p, qc, pv)

            # qc=3: p0 continues from the phase-1 prefetched groups, with
            # the v-projection zipped in as PE filler
            vproj_fillers = [(lambda t: lambda: emit_vproj(t))(tt)
                             for tt in range(KC)]
            pv = [aux_psp.tile([65, 512], F32, tag="aux",
                               name=f"pv{h01}") for h01 in (0, 1)]
            emit_attn(0, 3, pv, fillers=vproj_fillers)
            for p in (1, 2):
                pv = [aux_psp.tile([65, 512], F32, tag="aux",
                                   name=f"pv{h01}") for h01 in (0, 1)]
                emit_attn(p, 3, pv)
            emit_outproj(3)
            for p in range(NP):
                pv = [aux_psp.tile([65, 512], F32, tag="aux",
                                   name=f"pv{h01}") for h01 in (0, 1)]
                emit_attn(p, 2, pv)
            emit_outproj(2)

            # qc=1 and qc=0 tails: breadth-first across head-pairs to
            # overlap the short scores/exp/PV/norm latency chains; qc=0's
            # scores/exps run under qc=1's PV phase and outproj(1) fills
            # the PE during qc=0's exps
            helds1 = [[(pv_fn, sc_fn()) for sc_fn, pv_fn in attn_units(p, 1)]
                      for p in range(NP)]
            for p in range(NP):
                pv = [aux_psp.tile([65, 512], F32, tag="aux",
                                   name=f"pv{h01}") for h01 in (0, 1)]
                for pv_fn, pts in helds1[p]:
                    pv_fn(pts, pv)
                emit_norm(p, 1, pv, tail=True)
            helds0 = [[(pv_fn, sc_fn()) for sc_fn, pv_fn in attn_units(p, 0)]
                      for p in range(NP)]
            emit_outproj(1)
            for p in range(NP):
                pv = [aux_psp.tile([65, 512], F32, tag="aux",
                                   name=f"pv{h01}") for h01 in (0, 1)]
                for pv_fn, pts in helds0[p]:
                    pv_fn(pts, pv)
                emit_norm(p, 0, pv, tail=True)
            emit_outproj(0, tail=True)

    nc.compile()
    return nc


# make mybir importable inside _build_program's nested scopes
from concourse import mybir  # noqa: E402


def _get_compiled():
    global _COMPILED
    if _COMPILED is None:
        _COMPILED = _build_program()
    return _COMPILED


def _make_in_maps(inputs):
    import ml_dtypes

    BF = ml_dtypes.bfloat16
    x = np.asarray(inputs["x"], dtype=np.float32)
    wq = np.asarray(inputs["wq"], dtype=np.float32).astype(BF)
    wk = np.asarray(inputs["wk"], dtype=np.float32).astype(BF)
    wv = np.asarray(inputs["wv"], dtype=np.float32).astype(BF)
    wo = np.asarray(inputs["wo"], dtype=np.float32).astype(BF)

    ccat, scat = _rope_tables()
    r2t = _rot_matrix()
    m = np.arange(128)
    utri = (m[:, None] <= m[None, :]).astype(BF)
    eband = np.zeros((128, 128), dtype=np.float32)
    eband[np.arange(1, 128), np.arange(127)] = -1e9
    eband = eband.astype(BF)

    xTs = [np.ascontiguousarray(x[b].T.astype(BF)) for b in range(B)]
    in_maps = []
    for c in range(8):
        b, g = c // 2, c % 2
        sl = slice(g * HPC * D, (g + 1) * HPC * D)
        in_maps.append(dict(
            xT=xTs[b],
            wq=np.ascontiguousarray(wq[:, sl]),
            wk=np.ascontiguousarray(wk[:, sl]),
            wv=np.ascontiguousarray(wv[:, sl]),
            wo=np.ascontiguousarray(wo[sl, :]),
            ccat=ccat, scat=scat, r2t=r2t, utri=utri, eband=eband,
        ))
    return in_maps


def kernel(x, wq, wk, wv, wo, mask):
    """Full inputs in, full output out. Shards across 8 NeuronCores internally.

    The mask input is the standard causal mask produced by setup_inputs();
    causality is implemented directly on-device.
    """
    from concourse.bass_utils import run_bass_kernel_spmd

    in_maps = _make_in_maps(dict(x=x, wq=wq, wk=wk, wv=wv, wo=wo))

    nc = _get_compiled()
    res = run_bass_kernel_spmd(nc, in_maps, list(range(8)))
    out = np.empty((B, T, C), dtype=np.float32)
    for b in range(B):
        out[b] = res.results[2 * b]["y"] + res.results[2 * b + 1]["y"]
    return out

